# revision 5
# baseline (speedup 1.0000x reference)
"""Causal MHA (B=2, S=2048, D=2048, H=16) on 8 trn2 NeuronCores.

Sharding: tensor-parallel over heads. Each core computes QKV + RoPE + causal
SDPA for H/8 heads end-to-end, then an AllToAll per head redistributes
attention outputs from head-sharded to token-sharded layout, and each core
computes the full out-projection for its 1/8 token slice.

Attention runs head-outer so head 0's AllToAll fires at the attention
midpoint; output tiles bounce to DRAM per (b,jq) tile as they complete so
each AllToAll fires right after its head's last tile. The out-projection
accumulates all head-0 features into SBUF partials (pass A, hiding the
second AllToAll's rendezvous+wire), leaving only the head-1 features (pass
B) plus an add on the tail.

Within each QKV token tile all q/k matmuls are issued before the v matmuls
so RoPE (DVE) starts at ~2/3 of the tile's PE window instead of at its end;
this keeps DVE caught up and releases the q/k PSUM banks attention's first
scores matmuls recycle right when phase 1's matmuls end.

Layouts (partition dim = 128):
  xT      [128, T*DCH] fp16, host-rearranged so each token tile is one DMA
  q/k     per (hc,b) [128, S] fp16; per-head feature rows permuted
          [even;odd] so RoPE's pair rotation becomes a partition swap done
          with two partition-shifted Activation copies
  v       per b [128, 2*S] fp16 token-major, column block (256*g + 128*hc)
          matching the QKV v-psum bank layout: ONE [128,512] copy per bank
  scores  S^T tiles [tk=128, tq=512] f32 PSUM; diagonal tiles trimmed to
          their causally-valid columns [128*m:512] in the scores matmul,
          exp, mask multiply, l accumulation and AV matmul (the full-width
          m=0 chunk is processed last and carries stop=True); exp -> P fp16
          on Scalar; masks multiplicative fp16 on DVE; softmax denominator
          accumulated on DVE in fp16 + one ones-matmul per block
  out-proj: w_out fp16 fully preloaded into SBUF during attention; pass A
          parks f32 partials in SBUF, pass B adds head-1 features and emits
          fp16 output tiles.
"""

import numpy as np

import concourse.bass as bass
import concourse.bacc as bacc
import concourse.mybir as mybir
import concourse.tile as tile
from concourse import bass_utils

F32 = mybir.dt.float32
F16 = mybir.dt.float16


class Cfg:
    def __init__(self, B, S, D, H, NC=8):
        self.B, self.S, self.D, self.H, self.NC = B, S, D, H, NC
        self.DK = D // H
        assert self.DK == 128, "kernel assumes head dim 128"
        self.T = B * S                 # tokens, b-major
        self.HPC = H // NC             # heads per core
        self.FPC = self.HPC * self.DK  # features per core (q or k or v)
        self.W3 = 3 * self.FPC
        self.DCH = D // 128            # contraction chunks
        self.TT = 512                  # qkv token tile
        self.NTT = self.T // self.TT
        self.TQ = 512                  # attention tq tile
        self.SQT = S // self.TQ        # tq tiles per batch
        self.TPC = self.T // NC        # tokens per core for out-proj
        self.NTI = self.TPC // 128     # out-proj token chunks per core
        self.NJS = D // 512            # out-proj j tiles (512 wide)
        self.SCALE = float(1.0 / np.sqrt(self.DK))


FULL = Cfg(B=2, S=2048, D=2048, H=16, NC=8)


# --------------------------------------------------------------------------
# host-side prep
# --------------------------------------------------------------------------

def host_prep(cfg, x, w_qkv, w_out, cos, sin):
    B, S, D, H, NC = cfg.B, cfg.S, cfg.D, cfg.H, cfg.NC
    DK, T, HPC, FPC = cfg.DK, cfg.T, cfg.HPC, cfg.FPC

    x = np.asarray(x, dtype=np.float32)
    w_qkv = np.asarray(w_qkv, dtype=np.float32)
    w_out = np.asarray(w_out, dtype=np.float32)
    cos = np.asarray(cos, dtype=np.float32)
    sin = np.asarray(sin, dtype=np.float32)

    # xT rearranged so each token tile tt is ONE contiguous [128, DCH*TT]
    # DMA: xP[p, (tt*DCH + dc)*TT + c] = xT[128*dc + p, TT*tt + c]
    xT = x.reshape(T, D).T.astype(np.float16)                      # [D, T]
    NTT, DCH, TT = cfg.NTT, cfg.DCH, cfg.TT
    xP = np.ascontiguousarray(
        xT.reshape(DCH, 128, NTT, TT).transpose(1, 2, 0, 3).reshape(128, T * DCH)
    )

    # per-head [even; odd] feature permutation for q/k
    perm = np.concatenate([np.arange(0, DK, 2), np.arange(1, DK, 2)])
    w_q, w_k, w_v = w_qkv[0:D], w_qkv[D:2 * D], w_qkv[2 * D:3 * D]

    wqkvT = []
    for c in range(NC):
        rows = slice(FPC * c, FPC * (c + 1))
        wq = w_q[rows].reshape(HPC, DK, D)[:, perm, :].reshape(FPC, D)
        wk = w_k[rows].reshape(HPC, DK, D)[:, perm, :].reshape(FPC, D)
        wv = w_v[rows]
        wqkvT.append(np.ascontiguousarray(
            np.concatenate([wq, wk, wv], axis=0).T.astype(np.float16)))

    cosT = np.tile(cos.T, (1, B))                                  # [64, T]
    sinT = np.tile(sin.T, (1, B))
    cosF = np.ascontiguousarray(np.concatenate([cosT, cosT], axis=0), dtype=np.float32)
    # pre-swapped sin so rope's swap folds into partition-shifted copies:
    # swap(z)*[-s;s] == swap(z*[s;-s])
    sinF = np.ascontiguousarray(np.concatenate([sinT, -sinT], axis=0), dtype=np.float32)

    # multiplicative causal masks for S^T diagonal tiles [128, 4*TQ], fp16
    i = np.arange(128)[:, None]
    j = np.arange(cfg.TQ)[None, :]
    masks = np.concatenate(
        [np.where(i <= j - 128 * m, 1.0, 0.0).astype(np.float16) for m in range(4)],
        axis=1,
    )
    masks = np.ascontiguousarray(masks)

    # w_out^T rearranged so each 512-wide j-slice is ONE contiguous DMA:
    # woutP[p, (js*DCH + fc)*512 + c] = woutT[128*fc + p, 512*js + c]
    woutT = w_out.T.astype(np.float16)                             # [D(f), D(j)]
    NJS = cfg.NJS
    woutP = np.ascontiguousarray(
        woutT.reshape(DCH, 128, NJS, 512).transpose(1, 2, 0, 3).reshape(128, D * DCH)
    )

    ones = np.ones((128, 128), dtype=np.float16)
    shared = dict(xT=xP, cosF=cosF, sinF=sinF, masks=masks, ones=ones, woutT=woutP)
    return shared, wqkvT


# --------------------------------------------------------------------------
# device program
# --------------------------------------------------------------------------

def build_program(cfg):
    nc = bacc.Bacc(
        "TRN2",
        target_bir_lowering=False,
        debug=False,
        num_devices=cfg.NC,
    )

    xT_d = nc.dram_tensor("xT", [128, cfg.T * cfg.DCH], F16, kind="ExternalInput").ap()
    wqkvT_d = nc.dram_tensor("wqkvT", [cfg.D, cfg.W3], F16, kind="ExternalInput").ap()
    cosF_d = nc.dram_tensor("cosF", [128, cfg.T], F32, kind="ExternalInput").ap()
    sinF_d = nc.dram_tensor("sinF", [128, cfg.T], F32, kind="ExternalInput").ap()
    masks_d = nc.dram_tensor("masks", [128, 4 * cfg.TQ], F16, kind="ExternalInput").ap()
    ones_d = nc.dram_tensor("ones", [128, 128], F16, kind="ExternalInput").ap()
    woutT_d = nc.dram_tensor("woutT", [128, cfg.D * cfg.DCH], F16, kind="ExternalInput").ap()
    out_d = nc.dram_tensor("out", [cfg.TPC, cfg.D], F16, kind="ExternalOutput").ap()

    with tile.TileContext(nc) as tc:
        _build_body(
            nc, tc, cfg,
            xT_d, wqkvT_d, cosF_d, sinF_d, masks_d, ones_d, woutT_d, out_d,
        )

    nc.compile()
    return nc


def _phase1_qkv_rope(nc, tc, cfg, xT_d, wqkvT_d, cosF_d, sinF_d, q_t, k_t, v_t):
    T, HPC, FPC, W3 = cfg.T, cfg.HPC, cfg.FPC, cfg.W3
    DCH, NTT, TT = cfg.DCH, cfg.NTT, cfg.TT

    with (
        tc.tile_pool(name="wqkv", bufs=1) as wp,
        tc.tile_pool(name="xin", bufs=3) as xp,
        tc.tile_pool(name="csin", bufs=2) as csp,
        tc.tile_pool(name="ropet", bufs=2) as rtp,
        tc.tile_pool(name="pqk", bufs=6, space="PSUM") as pqkp,
        tc.tile_pool(name="pv", bufs=2, space="PSUM") as pvp,
    ):
        # per-dc weight tiles so the first matmul only waits on chunk 0
        w_ts = []
        for dc in range(DCH):
            w_t = wp.tile([128, W3], F16, name=f"w_{dc}")
            nc.scalar.dma_start(w_t[:], wqkvT_d[128 * dc:128 * (dc + 1), :])
            w_ts.append(w_t)

        for tt in range(NTT):
            b = tt // 4
            tl = tt % 4                 # tile index within batch
            # one contiguous DMA per token tile (host pre-arranged); tt=0 is
            # split per-dc so the first matmul starts as early as possible
            xfull = xp.tile([128, DCH * TT], F16, tag="x", name=f"x_{tt}")
            xbase = tt * DCH * TT
            if tt == 0:
                nc.sync.dma_start(xfull[:, 0:TT], xT_d[:, xbase:xbase + TT])
            # cos/sin on the GpSimd DMA queue to keep Sync clear for x
            cos_t = csp.tile([128, TT], F32, tag="cos")
            nc.gpsimd.dma_start(cos_t[:], cosF_d[:, TT * tt:TT * (tt + 1)])
            sin_t = csp.tile([128, TT], F32, tag="sin")
            nc.gpsimd.dma_start(sin_t[:], sinF_d[:, TT * tt:TT * (tt + 1)])
            if tt == 0:
                for dc in range(1, DCH):
                    nc.sync.dma_start(
                        xfull[:, TT * dc:TT * (dc + 1)],
                        xT_d[:, xbase + TT * dc:xbase + TT * (dc + 1)],
                    )
            else:
                nc.sync.dma_start(xfull[:], xT_d[:, xbase:xbase + DCH * TT])
            xts = [xfull[:, TT * dc:TT * (dc + 1)] for dc in range(DCH)]

            nqk = 2 * HPC
            pqs = [pqkp.tile([128, TT], F32, tag="qk", name=f"pq_{tt}_{i}")
                   for i in range(nqk)]
            # two v outputs (ci, ci+1) share one [128,512] PSUM bank
            pvt = [pvp.tile([128, 2 * FPC], F32, tag="v", name=f"pv_{tt}_{i}")
                   for i in range(2)]
            pvs = [pvt[ci // 2][:, FPC * (ci % 2):FPC * (ci % 2 + 1)]
                   for ci in range(4)]

            # all q/k matmuls FIRST so their psum stops land at ~2/3 of the
            # tile's PE window and RoPE (DVE) overlaps the v matmuls below
            for dc in range(DCH):
                first, last = dc == 0, dc == DCH - 1
                for oc in range(nqk):
                    wsl = w_ts[dc][:, 128 * oc:128 * (oc + 1)]
                    nc.tensor.matmul(
                        pqs[oc][:], wsl, xts[dc], start=first, stop=last
                    )
            for dc in range(DCH):
                first, last = dc == 0, dc == DCH - 1
                wv = w_ts[dc][:, 2 * FPC:W3]
                for ci in range(4):
                    lhs = xts[dc][:, 128 * ci:128 * (ci + 1)]
                    # start=True zeroes the WHOLE psum bank, so only the
                    # first group in each shared bank may issue it
                    nc.tensor.matmul(
                        pvs[ci][:], lhs, wv,
                        start=(first and ci % 2 == 0), stop=last,
                        skip_group_check=(ci % 2 == 1),
                    )

            # rope: z' = z*cosF + swap(z)*sinF. swap(z)*s == swap(z*s~) with
            # s~=swap(s); the swap itself is two partition-shifted
            # Activation copies (BIR allows shifted copy, not TensorTensor).
            for oc in range(nqk):
                hc = oc % HPC
                dst = q_t[(hc, b)] if oc < HPC else k_t[(hc, b)]
                z = pqs[oc]
                d0 = TT * tl
                t1 = rtp.tile([128, TT], F32, tag="t1", name=f"t1_{tt}_{oc}")
                u = rtp.tile([128, TT], F32, tag="u", name=f"u_{tt}_{oc}")
                usw = rtp.tile([128, TT], F32, tag="usw", name=f"usw_{tt}_{oc}")
                nc.vector.tensor_mul(t1[:], z[:], cos_t[:])
                nc.vector.tensor_mul(u[:], z[:], sin_t[:])
                nc.scalar.copy(usw[0:64, :], u[64:128, :])
                nc.scalar.copy(usw[64:128, :], u[0:64, :])
                nc.vector.tensor_add(dst[:, d0:d0 + TT], t1[:], usw[:])

            # v: each psum bank is [hc0 g, hc1 g, hc0 g+1, hc1 g+1] matching
            # v_t[b]'s (256*g_local + 128*hc) layout -> ONE copy per bank
            for i in range(2):
                gl = 4 * tl + 2 * i
                nc.scalar.copy(
                    v_t[b][:, 256 * gl:256 * gl + 512], pvt[i][:],
                )


def _phase2_attention(nc, tc, cfg, q_t, k_t, v_t, oT_t, masks_sb, ones_sb,
                      wout_sb, woutT_d, obs, a2a_outs):
    T, S, HPC, TQ, SQT = cfg.T, cfg.S, cfg.HPC, cfg.TQ, cfg.SQT
    DCH = cfg.DCH

    with (
        tc.tile_pool(name="pexp", bufs=8) as pep,
        tc.tile_pool(name="lacc", bufs=3) as lap,
        tc.tile_pool(name="linv", bufs=2) as lip,
        tc.tile_pool(name="pss", bufs=4, space="PSUM") as pssp,
        tc.tile_pool(name="pso", bufs=2, space="PSUM") as psop,
        tc.tile_pool(name="psl", bufs=2, space="PSUM") as pslp,
    ):
        # preload the full fp16 w_out while attention runs (4 batched DMAs
        # on the Scalar queue; the triggers cost ~30ns each amid the exps)
        for js in range(cfg.NJS):
            nc.scalar.dma_start(
                wout_sb[:, DCH * 512 * js:DCH * 512 * (js + 1)],
                woutT_d[:, DCH * 512 * js:DCH * 512 * (js + 1)],
            )

        onesr = ones_sb[:]
        for hc in range(HPC):
            for b in range(cfg.B):
                for jq in range(SQT):
                    o_ps = psop.tile([128, TQ], F32, tag="o",
                                     name=f"o_{hc}_{b}_{jq}")
                    l_ps = pslp.tile([128, TQ], F32, tag="l",
                                     name=f"l_{hc}_{b}_{jq}")
                    l_acc = lap.tile([128, TQ], F16, tag="la",
                                     name=f"la_{hc}_{b}_{jq}")
                    if jq == 0:
                        nc.vector.memset(l_acc[:], 0.0)
                    qsl = q_t[(hc, b)][:, TQ * jq:TQ * (jq + 1)]
                    # off-diagonal chunks first, then diagonal chunks
                    # m=3,2,1,0 so the full-width m=0 chunk is processed
                    # last and carries stop=True for the whole psum bank
                    cks = list(range(4 * jq)) + [4 * jq + m for m in (3, 2, 1, 0)]
                    first_ck = True
                    for ck in cks:
                        m = ck - 4 * jq
                        c0 = 128 * m if m >= 1 else 0
                        last_ck = (m == 0)
                        s_ps = pssp.tile([128, TQ], F32, tag="s",
                                         name=f"s_{hc}_{b}_{jq}_{ck}")
                        ksl = k_t[(hc, b)][:, 128 * ck:128 * (ck + 1)]
                        nc.tensor.matmul(
                            s_ps[:, c0:TQ], ksl, qsl[:, c0:TQ],
                            start=True, stop=True,
                        )
                        p_sb = pep.tile([128, TQ], F16, tag="p",
                                        name=f"p_{hc}_{b}_{jq}_{ck}")
                        nc.scalar.activation(
                            p_sb[:, c0:TQ], s_ps[:, c0:TQ],
                            mybir.ActivationFunctionType.Exp,
                            scale=cfg.SCALE,
                        )
                        if m >= 0:
                            nc.vector.tensor_mul(
                                p_sb[:, c0:TQ], p_sb[:, c0:TQ],
                                masks_sb[:, TQ * m + c0:TQ * (m + 1)],
                            )
                        if first_ck and jq > 0:
                            nc.vector.tensor_copy(l_acc[:], p_sb[:])
                        else:
                            nc.vector.tensor_add(
                                l_acc[:, c0:TQ], l_acc[:, c0:TQ],
                                p_sb[:, c0:TQ],
                            )
                        vsl = v_t[b][:, 256 * ck + 128 * hc:
                                     256 * ck + 128 * (hc + 1)]
                        nc.tensor.matmul(
                            o_ps[:, c0:TQ], vsl, p_sb[:, c0:TQ],
                            start=first_ck, stop=last_ck,
                        )
                        first_ck = False
                    nc.tensor.matmul(l_ps[:], onesr, l_acc[:],
                                     start=True, stop=True)
                    linv = lip.tile([128, TQ], F32, tag="li",
                                    name=f"li_{hc}_{b}_{jq}")
                    nc.vector.reciprocal_approx_fast(linv[:], l_ps[:])
                    nc.vector.tensor_mul(
                        oT_t[(hc, b)][:, TQ * jq:TQ * (jq + 1)],
                        o_ps[:], linv[:],
                    )
                    # bounce this tile to DRAM now so the head's AllToAll
                    # fires immediately after its last tile
                    s_idx = 4 * b + jq
                    nc.sync.dma_start(
                        obs[hc][128 * s_idx:128 * (s_idx + 1), :],
                        oT_t[(hc, b)][:, TQ * jq:TQ * (jq + 1)],
                    )
            # all 8 tiles of this head are bounced -> fire its AllToAll
            nc.gpsimd.collective_compute(
                "AllToAll",
                mybir.AluOpType.bypass,
                replica_groups=[list(range(cfg.NC))],
                ins=[obs[hc][:].opt()],
                outs=[a2a_outs[hc][:].opt()],
            )


def _phase3_outproj(nc, tc, cfg, wout_sb, a2a_outs, out_d):
    HPC, DCH, NJS, NTI = cfg.HPC, cfg.DCH, cfg.NJS, cfg.NTI

    with (
        tc.tile_pool(name="rhsp", bufs=1) as rhsp,
        tc.tile_pool(name="opart", bufs=1) as opp,
        tc.tile_pool(name="osb", bufs=3) as osbp,
        tc.tile_pool(name="pout", bufs=8, space="PSUM") as poutp,
    ):
        # gather each feature chunk; head-0 chunks (even fc, early A2A)
        # first so the later head-1 gathers can't head-of-line block them
        rhs_scr = rhsp.tile([128, DCH * cfg.TPC], F16, name="rhs_scr")
        rhs = [None] * DCH
        fcs_a = [fc for fc in range(DCH) if fc % HPC == 0]   # head 0 features
        fcs_b = [fc for fc in range(DCH) if fc % HPC != 0]   # head 1 features
        for fc in fcs_a + fcs_b:
            r_, hc = fc // HPC, fc % HPC
            sl = rhs_scr[:, cfg.TPC * fc:cfg.TPC * (fc + 1)]
            nc.sync.dma_start(sl, a2a_outs[hc][128 * r_:128 * (r_ + 1), :])
            rhs[fc] = sl

        tiles = [(js, ti) for js in range(NJS) for ti in range(NTI)]

        # pass A: accumulate ALL head-0 features for every output tile into
        # PSUM, park the partials in SBUF so the banks free up; hides the
        # second AllToAll's rendezvous + wire time
        opart = opp.tile([128, NJS * NTI * 512], F32, name="opart")
        for js, ti in tiles:
            ps = poutp.tile([128, 512], F32, tag="po", name=f"pa_{js}_{ti}")
            for i, fc in enumerate(fcs_a):
                nc.tensor.matmul(
                    ps[:],
                    rhs[fc][:, 128 * ti:128 * (ti + 1)],
                    wout_sb[:, (DCH * js + fc) * 512:
                            (DCH * js + fc) * 512 + 512],
                    start=(i == 0), stop=(i == len(fcs_a) - 1),
                )
            nc.vector.tensor_copy(
                opart[:, (NTI * js + ti) * 512:(NTI * js + ti + 1) * 512],
                ps[:],
            )

        # pass B: head-1 features + parked partial -> fp16 output tile
        for js, ti in tiles:
            ps = poutp.tile([128, 512], F32, tag="po", name=f"pb_{js}_{ti}")
            for i, fc in enumerate(fcs_b):
                nc.tensor.matmul(
                    ps[:],
                    rhs[fc][:, 128 * ti:128 * (ti + 1)],
                    wout_sb[:, (DCH * js + fc) * 512:
                            (DCH * js + fc) * 512 + 512],
                    start=(i == 0), stop=(i == len(fcs_b) - 1),
                )
            osb = osbp.tile([128, 512], F16, tag="ob", name=f"ob_{js}_{ti}")
            nc.vector.tensor_add(
                osb[:], ps[:],
                opart[:, (NTI * js + ti) * 512:(NTI * js + ti + 1) * 512],
            )
            nc.sync.dma_start(
                out_d[128 * ti:128 * (ti + 1), 512 * js:512 * (js + 1)],
                osb[:],
            )


def _build_body(nc, tc, cfg, xT_d, wqkvT_d, cosF_d, sinF_d, masks_d, ones_d,
                woutT_d, out_d):
    T, S, HPC, TQ = cfg.T, cfg.S, cfg.HPC, cfg.TQ

    with tc.tile_pool(name="const", bufs=1) as constp:
        ones_sb = constp.tile([128, 128], F16)
        nc.gpsimd.dma_start(ones_sb[:], ones_d[:])
        masks_sb = constp.tile([128, 4 * TQ], F16)
        nc.gpsimd.dma_start(masks_sb[:], masks_d[:])
        # warm the Exp activation table off the critical path
        warm = constp.tile([128, 1], F32, name="actwarm")
        nc.scalar.activation(
            warm[:], ones_sb[:, 0:1], mybir.ActivationFunctionType.Exp
        )

        with tc.tile_pool(name="qkvp", bufs=1) as qkvp:
            q_t, k_t, oT_t, v_t = {}, {}, {}, {}
            for hc in range(HPC):
                for b in range(cfg.B):
                    q_t[(hc, b)] = qkvp.tile([128, S], F16, name=f"q_{hc}_{b}")
                    k_t[(hc, b)] = qkvp.tile([128, S], F16, name=f"k_{hc}_{b}")
            for b in range(cfg.B):
                v_t[b] = qkvp.tile([128, HPC * S], F16, name=f"v_{b}")

            _phase1_qkv_rope(
                nc, tc, cfg, xT_d, wqkvT_d, cosF_d, sinF_d, q_t, k_t, v_t
            )

            with (
                tc.tile_pool(name="oT", bufs=1) as otp,
                tc.tile_pool(name="dram", bufs=1, space="DRAM") as dramp,
            ):
                for hc in range(HPC):
                    for b in range(cfg.B):
                        oT_t[(hc, b)] = otp.tile(
                            [128, S], F16, name=f"oT_{hc}_{b}")
                wout_sb = otp.tile([128, cfg.DCH * cfg.D], F16)

                obs, a2a_outs = [], []
                for hc in range(HPC):
                    obs.append(dramp.tile(
                        [cfg.NC * 128, cfg.TPC], F16, name=f"obounce{hc}"))
                    a2a_outs.append(dramp.tile(
                        [cfg.NC * 128, cfg.TPC], F16, name=f"a2a_out{hc}"))

                _phase2_attention(
                    nc, tc, cfg, q_t, k_t, v_t, oT_t, masks_sb, ones_sb,
                    wout_sb, woutT_d, obs, a2a_outs,
                )
                _phase3_outproj(nc, tc, cfg, wout_sb, a2a_outs, out_d)


# --------------------------------------------------------------------------
# host entry point
# --------------------------------------------------------------------------

_CACHE = {}


def _compiled(cfg):
    key = (cfg.B, cfg.S, cfg.D, cfg.H, cfg.NC)
    if key not in _CACHE:
        _CACHE[key] = build_program(cfg)
    return _CACHE[key]


def make_in_maps(cfg, inputs):
    shared, wqkvT = host_prep(
        cfg, inputs["x"], inputs["w_qkv"], inputs["w_out"],
        inputs["cos"], inputs["sin"],
    )
    return [{**shared, "wqkvT": wqkvT[c]} for c in range(cfg.NC)]


def assemble(cfg, results):
    out = np.concatenate([results[c]["out"] for c in range(cfg.NC)], axis=0)
    return out.reshape(cfg.B, cfg.S, cfg.D).astype(np.float32)


def kernel(x, w_qkv, w_out, cos, sin):
    cfg = FULL
    nc = _compiled(cfg)
    in_maps = make_in_maps(cfg, dict(x=x, w_qkv=w_qkv, w_out=w_out, cos=cos, sin=sin))
    res = bass_utils.run_bass_kernel_spmd(nc, in_maps, core_ids=list(range(cfg.NC)))
    return assemble(cfg, res.results)


# revision 11
# speedup vs baseline: 1.0176x; 1.0176x over previous
"""Causal MHA (B=2, S=2048, D=2048, H=16) on 8 trn2 NeuronCores.

Sharding: tensor-parallel over heads. Each core computes QKV + RoPE + causal
SDPA for H/8 heads end-to-end, then an AllToAll per head redistributes
attention outputs from head-sharded to token-sharded layout, and each core
computes the full out-projection for its 1/8 token slice.

Attention runs head-outer so head 0's AllToAll fires at the attention
midpoint; output tiles bounce to DRAM per (b,jq) tile as they complete so
each AllToAll fires right after its head's last tile. The out-projection
accumulates all head-0 features into SBUF partials (pass A, hiding the
second AllToAll's rendezvous+wire), leaving only the head-1 features (pass
B) plus an add on the tail.

Within each QKV token tile all q/k matmuls are issued before the v matmuls
so RoPE (DVE) starts at ~2/3 of the tile's PE window instead of at its end;
this keeps DVE caught up and releases the q/k PSUM banks attention's first
scores matmuls recycle right when phase 1's matmuls end.

Layouts (partition dim = 128):
  xT      [128, T*DCH] fp16, host-rearranged so each token tile is one DMA
  q/k     per (hc,b) [128, S] fp16; per-head feature rows permuted
          [even;odd] so RoPE's pair rotation becomes a partition swap done
          with two partition-shifted Activation copies
  v       per b [128, 2*S] fp16 token-major, column block (256*g + 128*hc)
          matching the QKV v-psum bank layout: ONE [128,512] copy per bank
  scores  S^T tiles [tk=128, tq=512] f32 PSUM; diagonal tiles trimmed to
          their causally-valid columns [128*m:512] in the scores matmul,
          exp, mask multiply, l accumulation and AV matmul (the full-width
          m=0 chunk is processed last and carries stop=True); exp -> P fp16
          on Scalar; masks multiplicative fp16 on DVE; softmax denominator
          accumulated on DVE in fp16 + one ones-matmul per block
  out-proj: w_out fp16 fully preloaded into SBUF during attention; pass A
          parks f32 partials in SBUF, pass B adds head-1 features and emits
          fp16 output tiles.
"""

import numpy as np

import concourse.bass as bass
import concourse.bacc as bacc
import concourse.mybir as mybir
import concourse.tile as tile
from concourse import bass_utils

F32 = mybir.dt.float32
F16 = mybir.dt.float16


class Cfg:
    def __init__(self, B, S, D, H, NC=8):
        self.B, self.S, self.D, self.H, self.NC = B, S, D, H, NC
        self.DK = D // H
        assert self.DK == 128, "kernel assumes head dim 128"
        self.T = B * S                 # tokens, b-major
        self.HPC = H // NC             # heads per core
        self.FPC = self.HPC * self.DK  # features per core (q or k or v)
        self.W3 = 3 * self.FPC
        self.DCH = D // 128            # contraction chunks
        self.TT = 512                  # qkv token tile
        self.NTT = self.T // self.TT
        self.TQ = 512                  # attention tq tile
        self.SQT = S // self.TQ        # tq tiles per batch
        self.TPC = self.T // NC        # tokens per core for out-proj
        self.NTI = self.TPC // 128     # out-proj token chunks per core
        self.NJS = D // 512            # out-proj j tiles (512 wide)
        self.SCALE = float(1.0 / np.sqrt(self.DK))


FULL = Cfg(B=2, S=2048, D=2048, H=16, NC=8)


# --------------------------------------------------------------------------
# host-side prep
# --------------------------------------------------------------------------

def host_prep(cfg, x, w_qkv, w_out, cos, sin):
    B, S, D, H, NC = cfg.B, cfg.S, cfg.D, cfg.H, cfg.NC
    DK, T, HPC, FPC = cfg.DK, cfg.T, cfg.HPC, cfg.FPC

    x = np.asarray(x, dtype=np.float32)
    w_qkv = np.asarray(w_qkv, dtype=np.float32)
    w_out = np.asarray(w_out, dtype=np.float32)
    cos = np.asarray(cos, dtype=np.float32)
    sin = np.asarray(sin, dtype=np.float32)

    # xT rearranged so each token tile tt is ONE contiguous [128, DCH*TT]
    # DMA: xP[p, (tt*DCH + dc)*TT + c] = xT[128*dc + p, TT*tt + c]
    xT = x.reshape(T, D).T.astype(np.float16)                      # [D, T]
    NTT, DCH, TT = cfg.NTT, cfg.DCH, cfg.TT
    xP = np.ascontiguousarray(
        xT.reshape(DCH, 128, NTT, TT).transpose(1, 2, 0, 3).reshape(128, T * DCH)
    )

    # per-head [even; odd] feature permutation for q/k
    perm = np.concatenate([np.arange(0, DK, 2), np.arange(1, DK, 2)])
    w_q, w_k, w_v = w_qkv[0:D], w_qkv[D:2 * D], w_qkv[2 * D:3 * D]

    wqkvT = []
    for c in range(NC):
        rows = slice(FPC * c, FPC * (c + 1))
        wq = w_q[rows].reshape(HPC, DK, D)[:, perm, :].reshape(FPC, D)
        wk = w_k[rows].reshape(HPC, DK, D)[:, perm, :].reshape(FPC, D)
        wv = w_v[rows]
        wqkvT.append(np.ascontiguousarray(
            np.concatenate([wq, wk, wv], axis=0).T.astype(np.float16)))

    cosT = np.tile(cos.T, (1, B))                                  # [64, T]
    sinT = np.tile(sin.T, (1, B))
    cosF = np.ascontiguousarray(np.concatenate([cosT, cosT], axis=0), dtype=np.float32)
    # pre-swapped sin so rope's swap folds into partition-shifted copies:
    # swap(z)*[-s;s] == swap(z*[s;-s])
    sinF = np.ascontiguousarray(np.concatenate([sinT, -sinT], axis=0), dtype=np.float32)

    # multiplicative causal masks for S^T diagonal tiles [128, 4*TQ], fp16
    i = np.arange(128)[:, None]
    j = np.arange(cfg.TQ)[None, :]
    masks = np.concatenate(
        [np.where(i <= j - 128 * m, 1.0, 0.0).astype(np.float16) for m in range(4)],
        axis=1,
    )
    masks = np.ascontiguousarray(masks)

    # w_out^T rearranged so each 512-wide j-slice is ONE contiguous DMA:
    # woutP[p, (js*DCH + fc)*512 + c] = woutT[128*fc + p, 512*js + c]
    woutT = w_out.T.astype(np.float16)                             # [D(f), D(j)]
    NJS = cfg.NJS
    woutP = np.ascontiguousarray(
        woutT.reshape(DCH, 128, NJS, 512).transpose(1, 2, 0, 3).reshape(128, D * DCH)
    )

    ones = np.ones((128, 128), dtype=np.float16)
    shared = dict(xT=xP, cosF=cosF, sinF=sinF, masks=masks, ones=ones, woutT=woutP)
    return shared, wqkvT


# --------------------------------------------------------------------------
# device program
# --------------------------------------------------------------------------

def build_program(cfg):
    nc = bacc.Bacc(
        "TRN2",
        target_bir_lowering=False,
        debug=False,
        num_devices=cfg.NC,
    )

    xT_d = nc.dram_tensor("xT", [128, cfg.T * cfg.DCH], F16, kind="ExternalInput").ap()
    wqkvT_d = nc.dram_tensor("wqkvT", [cfg.D, cfg.W3], F16, kind="ExternalInput").ap()
    cosF_d = nc.dram_tensor("cosF", [128, cfg.T], F32, kind="ExternalInput").ap()
    sinF_d = nc.dram_tensor("sinF", [128, cfg.T], F32, kind="ExternalInput").ap()
    masks_d = nc.dram_tensor("masks", [128, 4 * cfg.TQ], F16, kind="ExternalInput").ap()
    ones_d = nc.dram_tensor("ones", [128, 128], F16, kind="ExternalInput").ap()
    woutT_d = nc.dram_tensor("woutT", [128, cfg.D * cfg.DCH], F16, kind="ExternalInput").ap()
    out_d = nc.dram_tensor("out", [cfg.TPC, cfg.D], F16, kind="ExternalOutput").ap()

    with tile.TileContext(nc) as tc:
        _build_body(
            nc, tc, cfg,
            xT_d, wqkvT_d, cosF_d, sinF_d, masks_d, ones_d, woutT_d, out_d,
        )

    nc.compile()
    return nc


def _phase1_qkv_rope(nc, tc, cfg, xT_d, wqkvT_d, cosF_d, sinF_d, q_t, k_t, v_t):
    T, HPC, FPC, W3 = cfg.T, cfg.HPC, cfg.FPC, cfg.W3
    DCH, NTT, TT = cfg.DCH, cfg.NTT, cfg.TT

    with (
        tc.tile_pool(name="wqkv", bufs=1) as wp,
        tc.tile_pool(name="xin", bufs=3) as xp,
        tc.tile_pool(name="csin", bufs=2) as csp,
        tc.tile_pool(name="ropet", bufs=2) as rtp,
        tc.tile_pool(name="pqk", bufs=6, space="PSUM") as pqkp,
        tc.tile_pool(name="pv", bufs=2, space="PSUM") as pvp,
    ):
        # per-dc weight tiles so the first matmul only waits on chunk 0
        w_ts = []
        for dc in range(DCH):
            w_t = wp.tile([128, W3], F16, name=f"w_{dc}")
            nc.scalar.dma_start(w_t[:], wqkvT_d[128 * dc:128 * (dc + 1), :])
            w_ts.append(w_t)

        for tt in range(NTT):
            b = tt // 4
            tl = tt % 4                 # tile index within batch
            # one contiguous DMA per token tile (host pre-arranged); tt=0 is
            # split per-dc so the first matmul starts as early as possible
            xfull = xp.tile([128, DCH * TT], F16, tag="x", name=f"x_{tt}")
            xbase = tt * DCH * TT
            if tt == 0:
                nc.sync.dma_start(xfull[:, 0:TT], xT_d[:, xbase:xbase + TT])
            cos_t = csp.tile([128, TT], F32, tag="cos")
            nc.sync.dma_start(cos_t[:], cosF_d[:, TT * tt:TT * (tt + 1)])
            sin_t = csp.tile([128, TT], F32, tag="sin")
            nc.sync.dma_start(sin_t[:], sinF_d[:, TT * tt:TT * (tt + 1)])
            if tt == 0:
                for dc in range(1, DCH):
                    nc.sync.dma_start(
                        xfull[:, TT * dc:TT * (dc + 1)],
                        xT_d[:, xbase + TT * dc:xbase + TT * (dc + 1)],
                    )
            else:
                nc.sync.dma_start(xfull[:], xT_d[:, xbase:xbase + DCH * TT])
            xts = [xfull[:, TT * dc:TT * (dc + 1)] for dc in range(DCH)]

            nqk = 2 * HPC
            pqs = [pqkp.tile([128, TT], F32, tag="qk", name=f"pq_{tt}_{i}")
                   for i in range(nqk)]
            # two v outputs (ci, ci+1) share one [128,512] PSUM bank
            pvt = [pvp.tile([128, 2 * FPC], F32, tag="v", name=f"pv_{tt}_{i}")
                   for i in range(2)]
            pvs = [pvt[ci // 2][:, FPC * (ci % 2):FPC * (ci % 2 + 1)]
                   for ci in range(4)]

            # all q/k matmuls FIRST so their psum stops land at ~2/3 of the
            # tile's PE window and RoPE (DVE) overlaps the v matmuls below
            for dc in range(DCH):
                first, last = dc == 0, dc == DCH - 1
                for oc in range(nqk):
                    wsl = w_ts[dc][:, 128 * oc:128 * (oc + 1)]
                    nc.tensor.matmul(
                        pqs[oc][:], wsl, xts[dc], start=first, stop=last
                    )
            for dc in range(DCH):
                first, last = dc == 0, dc == DCH - 1
                wv = w_ts[dc][:, 2 * FPC:W3]
                for ci in range(4):
                    lhs = xts[dc][:, 128 * ci:128 * (ci + 1)]
                    # start=True zeroes the WHOLE psum bank, so only the
                    # first group in each shared bank may issue it
                    nc.tensor.matmul(
                        pvs[ci][:], lhs, wv,
                        start=(first and ci % 2 == 0), stop=last,
                        skip_group_check=(ci % 2 == 1),
                    )

            # rope: z' = z*cosF + swap(z)*sinF. swap(z)*s == swap(z*s~) with
            # s~=swap(s); the swap itself is two partition-shifted
            # Activation copies (BIR allows shifted copy, not TensorTensor).
            for oc in range(nqk):
                hc = oc % HPC
                dst = q_t[(hc, b)] if oc < HPC else k_t[(hc, b)]
                z = pqs[oc]
                d0 = TT * tl
                t1 = rtp.tile([128, TT], F32, tag="t1", name=f"t1_{tt}_{oc}")
                u = rtp.tile([128, TT], F32, tag="u", name=f"u_{tt}_{oc}")
                usw = rtp.tile([128, TT], F32, tag="usw", name=f"usw_{tt}_{oc}")
                nc.vector.tensor_mul(t1[:], z[:], cos_t[:])
                nc.vector.tensor_mul(u[:], z[:], sin_t[:])
                nc.scalar.copy(usw[0:64, :], u[64:128, :])
                nc.scalar.copy(usw[64:128, :], u[0:64, :])
                nc.vector.tensor_add(dst[:, d0:d0 + TT], t1[:], usw[:])

            # v: each psum bank is [hc0 g, hc1 g, hc0 g+1, hc1 g+1] matching
            # v_t[b]'s (256*g_local + 128*hc) layout -> ONE copy per bank
            for i in range(2):
                gl = 4 * tl + 2 * i
                nc.scalar.copy(
                    v_t[b][:, 256 * gl:256 * gl + 512], pvt[i][:],
                )


def _phase2_attention(nc, tc, cfg, q_t, k_t, v_t, oT_t, masks_sb, ones_sb,
                      wout_sb, woutT_d, obs, a2a_outs, rhs_gather):
    T, S, HPC, TQ, SQT = cfg.T, cfg.S, cfg.HPC, cfg.TQ, cfg.SQT
    DCH = cfg.DCH

    with (
        tc.tile_pool(name="pexp", bufs=8) as pep,
        tc.tile_pool(name="lacc", bufs=3) as lap,
        tc.tile_pool(name="linv", bufs=2) as lip,
        tc.tile_pool(name="pss", bufs=4, space="PSUM") as pssp,
        tc.tile_pool(name="pso", bufs=2, space="PSUM") as psop,
        tc.tile_pool(name="psl", bufs=2, space="PSUM") as pslp,
    ):
        # preload the full fp16 w_out while attention runs (4 batched DMAs
        # on the Scalar queue; the triggers cost ~30ns each amid the exps)
        for js in range(cfg.NJS):
            nc.scalar.dma_start(
                wout_sb[:, DCH * 512 * js:DCH * 512 * (js + 1)],
                woutT_d[:, DCH * 512 * js:DCH * 512 * (js + 1)],
            )

        onesr = ones_sb[:]

        # The PE executes its queue in order, so an AV matmul waiting on its
        # exp would stall later, already-runnable scores matmuls. Software-
        # pipeline the PE issue order: scores run LOOKAHEAD chunks ahead of
        # the AV matmuls, across block boundaries.
        LOOKAHEAD = 2
        pend_av = []

        def drain_av(keep):
            while len(pend_av) > keep:
                pend_av.pop(0)()

        for hc in range(HPC):
            for b in range(cfg.B):
                for jq in range(SQT):
                    o_ps = psop.tile([128, TQ], F32, tag="o",
                                     name=f"o_{hc}_{b}_{jq}")
                    l_ps = pslp.tile([128, TQ], F32, tag="l",
                                     name=f"l_{hc}_{b}_{jq}")
                    l_acc = lap.tile([128, TQ], F16, tag="la",
                                     name=f"la_{hc}_{b}_{jq}")
                    if jq == 0:
                        nc.vector.memset(l_acc[:], 0.0)
                    qsl = q_t[(hc, b)][:, TQ * jq:TQ * (jq + 1)]
                    # off-diagonal chunks first, then diagonal chunks
                    # m=3,2,1,0 so the full-width m=0 chunk is processed
                    # last and carries stop=True for the whole psum bank
                    cks = list(range(4 * jq)) + [4 * jq + m for m in (3, 2, 1, 0)]
                    for idx, ck in enumerate(cks):
                        m = ck - 4 * jq
                        c0 = 128 * m if m >= 1 else 0
                        last_ck = (m == 0)
                        first_ck = (idx == 0)
                        s_ps = pssp.tile([128, TQ], F32, tag="s",
                                         name=f"s_{hc}_{b}_{jq}_{ck}")
                        ksl = k_t[(hc, b)][:, 128 * ck:128 * (ck + 1)]
                        nc.tensor.matmul(
                            s_ps[:, c0:TQ], ksl, qsl[:, c0:TQ],
                            start=True, stop=True,
                        )
                        p_sb = pep.tile([128, TQ], F16, tag="p",
                                        name=f"p_{hc}_{b}_{jq}_{ck}")
                        nc.scalar.activation(
                            p_sb[:, c0:TQ], s_ps[:, c0:TQ],
                            mybir.ActivationFunctionType.Exp,
                            scale=cfg.SCALE,
                        )
                        if m >= 0:
                            nc.vector.tensor_mul(
                                p_sb[:, c0:TQ], p_sb[:, c0:TQ],
                                masks_sb[:, TQ * m + c0:TQ * (m + 1)],
                            )
                        if first_ck and jq > 0:
                            nc.vector.tensor_copy(l_acc[:], p_sb[:])
                        else:
                            nc.vector.tensor_add(
                                l_acc[:, c0:TQ], l_acc[:, c0:TQ],
                                p_sb[:, c0:TQ],
                            )
                        vsl = v_t[b][:, 256 * ck + 128 * hc:
                                     256 * ck + 128 * (hc + 1)]

                        def av(o_ps=o_ps, vsl=vsl, p_sb=p_sb, c0=c0,
                               first_ck=first_ck, last_ck=last_ck):
                            nc.tensor.matmul(
                                o_ps[:, c0:TQ], vsl, p_sb[:, c0:TQ],
                                start=first_ck, stop=last_ck,
                            )
                        pend_av.append(av)
                        drain_av(LOOKAHEAD)
                    # the omult below must follow the stop-AV in issue order
                    drain_av(0)
                    nc.tensor.matmul(l_ps[:], onesr, l_acc[:],
                                     start=True, stop=True)
                    linv = lip.tile([128, TQ], F32, tag="li",
                                    name=f"li_{hc}_{b}_{jq}")
                    nc.vector.reciprocal_approx_fast(linv[:], l_ps[:])
                    nc.vector.tensor_mul(
                        oT_t[(hc, b)][:, TQ * jq:TQ * (jq + 1)],
                        o_ps[:], linv[:],
                    )
                    # bounce this tile to DRAM now so the head's AllToAll
                    # fires immediately after its last tile
                    s_idx = 4 * b + jq
                    nc.sync.dma_start(
                        obs[hc][128 * s_idx:128 * (s_idx + 1), :],
                        oT_t[(hc, b)][:, TQ * jq:TQ * (jq + 1)],
                    )
            drain_av(0)
            # all 8 tiles of this head are bounced -> fire its AllToAll
            nc.gpsimd.collective_compute(
                "AllToAll",
                mybir.AluOpType.bypass,
                replica_groups=[list(range(cfg.NC))],
                ins=[obs[hc][:].opt()],
                outs=[a2a_outs[hc][:].opt()],
            )
            # issue this head's out-proj gathers now: they wait on the A2A
            # sem, and anything queued behind them on Sync (the other head's
            # bounces) completes long before that A2A's data is needed
            rhs_gather(hc)


def _phase3_outproj(nc, tc, cfg, wout_sb, rhs, out_d):
    HPC, DCH, NJS, NTI = cfg.HPC, cfg.DCH, cfg.NJS, cfg.NTI

    with (
        tc.tile_pool(name="opart", bufs=1) as opp,
        tc.tile_pool(name="osb", bufs=3) as osbp,
        tc.tile_pool(name="pout", bufs=8, space="PSUM") as poutp,
    ):
        fcs_a = [fc for fc in range(DCH) if fc % HPC == 0]   # head 0 features
        fcs_b = [fc for fc in range(DCH) if fc % HPC != 0]   # head 1 features
        tiles = [(js, ti) for js in range(NJS) for ti in range(NTI)]

        # pass A: accumulate ALL head-0 features for every output tile into
        # PSUM, park the partials in SBUF so the banks free up; hides the
        # second AllToAll's rendezvous + wire time
        opart = opp.tile([128, NJS * NTI * 512], F32, name="opart")
        for js, ti in tiles:
            ps = poutp.tile([128, 512], F32, tag="po", name=f"pa_{js}_{ti}")
            for i, fc in enumerate(fcs_a):
                nc.tensor.matmul(
                    ps[:],
                    rhs[fc][:, 128 * ti:128 * (ti + 1)],
                    wout_sb[:, (DCH * js + fc) * 512:
                            (DCH * js + fc) * 512 + 512],
                    start=(i == 0), stop=(i == len(fcs_a) - 1),
                )
            nc.vector.tensor_copy(
                opart[:, (NTI * js + ti) * 512:(NTI * js + ti + 1) * 512],
                ps[:],
            )

        # pass B: head-1 features + parked partial -> fp16 output tile
        for js, ti in tiles:
            ps = poutp.tile([128, 512], F32, tag="po", name=f"pb_{js}_{ti}")
            for i, fc in enumerate(fcs_b):
                nc.tensor.matmul(
                    ps[:],
                    rhs[fc][:, 128 * ti:128 * (ti + 1)],
                    wout_sb[:, (DCH * js + fc) * 512:
                            (DCH * js + fc) * 512 + 512],
                    start=(i == 0), stop=(i == len(fcs_b) - 1),
                )
            osb = osbp.tile([128, 512], F16, tag="ob", name=f"ob_{js}_{ti}")
            nc.vector.tensor_add(
                osb[:], ps[:],
                opart[:, (NTI * js + ti) * 512:(NTI * js + ti + 1) * 512],
            )
            nc.sync.dma_start(
                out_d[128 * ti:128 * (ti + 1), 512 * js:512 * (js + 1)],
                osb[:],
            )


def _build_body(nc, tc, cfg, xT_d, wqkvT_d, cosF_d, sinF_d, masks_d, ones_d,
                woutT_d, out_d):
    T, S, HPC, TQ = cfg.T, cfg.S, cfg.HPC, cfg.TQ

    with tc.tile_pool(name="const", bufs=1) as constp:
        ones_sb = constp.tile([128, 128], F16)
        nc.gpsimd.dma_start(ones_sb[:], ones_d[:])
        masks_sb = constp.tile([128, 4 * TQ], F16)
        nc.gpsimd.dma_start(masks_sb[:], masks_d[:])
        # warm the Exp activation table off the critical path
        warm = constp.tile([128, 1], F32, name="actwarm")
        nc.scalar.activation(
            warm[:], ones_sb[:, 0:1], mybir.ActivationFunctionType.Exp
        )

        with tc.tile_pool(name="qkvp", bufs=1) as qkvp:
            q_t, k_t, oT_t, v_t = {}, {}, {}, {}
            for hc in range(HPC):
                for b in range(cfg.B):
                    q_t[(hc, b)] = qkvp.tile([128, S], F16, name=f"q_{hc}_{b}")
                    k_t[(hc, b)] = qkvp.tile([128, S], F16, name=f"k_{hc}_{b}")
            for b in range(cfg.B):
                v_t[b] = qkvp.tile([128, HPC * S], F16, name=f"v_{b}")

            _phase1_qkv_rope(
                nc, tc, cfg, xT_d, wqkvT_d, cosF_d, sinF_d, q_t, k_t, v_t
            )

            with (
                tc.tile_pool(name="oT", bufs=1) as otp,
                tc.tile_pool(name="dram", bufs=1, space="DRAM") as dramp,
            ):
                for hc in range(HPC):
                    for b in range(cfg.B):
                        oT_t[(hc, b)] = otp.tile(
                            [128, S], F16, name=f"oT_{hc}_{b}")
                wout_sb = otp.tile([128, cfg.DCH * cfg.D], F16)

                obs, a2a_outs = [], []
                for hc in range(HPC):
                    obs.append(dramp.tile(
                        [cfg.NC * 128, cfg.TPC], F16, name=f"obounce{hc}"))
                    a2a_outs.append(dramp.tile(
                        [cfg.NC * 128, cfg.TPC], F16, name=f"a2a_out{hc}"))

                with tc.tile_pool(name="rhsp", bufs=1) as rhsp:
                    rhs_scr = rhsp.tile(
                        [128, cfg.DCH * cfg.TPC], F16, name="rhs_scr")
                    rhs = [None] * cfg.DCH

                    def rhs_gather(hc):
                        for r_ in range(cfg.NC):
                            fc = r_ * HPC + hc
                            sl = rhs_scr[:, cfg.TPC * fc:cfg.TPC * (fc + 1)]
                            nc.sync.dma_start(
                                sl,
                                a2a_outs[hc][128 * r_:128 * (r_ + 1), :])
                            rhs[fc] = sl

                    _phase2_attention(
                        nc, tc, cfg, q_t, k_t, v_t, oT_t, masks_sb, ones_sb,
                        wout_sb, woutT_d, obs, a2a_outs, rhs_gather,
                    )
                    _phase3_outproj(nc, tc, cfg, wout_sb, rhs, out_d)


# --------------------------------------------------------------------------
# host entry point
# --------------------------------------------------------------------------

_CACHE = {}


def _compiled(cfg):
    key = (cfg.B, cfg.S, cfg.D, cfg.H, cfg.NC)
    if key not in _CACHE:
        _CACHE[key] = build_program(cfg)
    return _CACHE[key]


def make_in_maps(cfg, inputs):
    shared, wqkvT = host_prep(
        cfg, inputs["x"], inputs["w_qkv"], inputs["w_out"],
        inputs["cos"], inputs["sin"],
    )
    return [{**shared, "wqkvT": wqkvT[c]} for c in range(cfg.NC)]


def assemble(cfg, results):
    out = np.concatenate([results[c]["out"] for c in range(cfg.NC)], axis=0)
    return out.reshape(cfg.B, cfg.S, cfg.D).astype(np.float32)


def kernel(x, w_qkv, w_out, cos, sin):
    cfg = FULL
    nc = _compiled(cfg)
    in_maps = make_in_maps(cfg, dict(x=x, w_qkv=w_qkv, w_out=w_out, cos=cos, sin=sin))
    res = bass_utils.run_bass_kernel_spmd(nc, in_maps, core_ids=list(range(cfg.NC)))
    return assemble(cfg, res.results)


# revision 15
# speedup vs baseline: 1.0384x; 1.0204x over previous
"""Causal MHA (B=2, S=2048, D=2048, H=16) on 8 trn2 NeuronCores.

Sharding: tensor-parallel over heads. Each core computes QKV + RoPE + causal
SDPA for H/8 heads end-to-end, then an AllToAll per head redistributes
attention outputs from head-sharded to token-sharded layout, and each core
computes the full out-projection for its 1/8 token slice.

Attention runs head-outer so head 0's AllToAll fires at the attention
midpoint; output tiles bounce to DRAM per (b,jq) tile as they complete so
each AllToAll fires right after its head's last tile. The out-projection
accumulates all head-0 features into SBUF partials (pass A, hiding the
second AllToAll's rendezvous+wire), leaving only the head-1 features (pass
B) plus an add on the tail.

Within each QKV token tile all q/k matmuls are issued before the v matmuls
so RoPE (DVE) starts at ~2/3 of the tile's PE window instead of at its end;
this keeps DVE caught up and releases the q/k PSUM banks attention's first
scores matmuls recycle right when phase 1's matmuls end.

Layouts (partition dim = 128):
  xT      [128, T*DCH] fp16, host-rearranged so each token tile is one DMA
  q/k     per (hc,b) [128, S] fp16; per-head feature rows permuted
          [even;odd] so RoPE's pair rotation becomes a partition swap done
          with two partition-shifted Activation copies
  v       per b [128, 2*S] fp16 token-major, column block (256*g + 128*hc)
          matching the QKV v-psum bank layout: ONE [128,512] copy per bank
  scores  S^T tiles [tk=128, tq=512] f32 PSUM; diagonal tiles trimmed to
          their causally-valid columns [128*m:512] in the scores matmul,
          exp, mask multiply, l accumulation and AV matmul (the full-width
          m=0 chunk is processed last and carries stop=True); exp -> P fp16
          on Scalar; masks multiplicative fp16 on DVE; softmax denominator
          accumulated on DVE in fp16 + one ones-matmul per block
  out-proj: w_out fp16 fully preloaded into SBUF during attention; pass A
          parks f32 partials in SBUF, pass B adds head-1 features and emits
          fp16 output tiles.
"""

import numpy as np

import concourse.bass as bass
import concourse.bacc as bacc
import concourse.mybir as mybir
import concourse.tile as tile
from concourse import bass_utils

F32 = mybir.dt.float32
F16 = mybir.dt.float16


class Cfg:
    def __init__(self, B, S, D, H, NC=8):
        self.B, self.S, self.D, self.H, self.NC = B, S, D, H, NC
        self.DK = D // H
        assert self.DK == 128, "kernel assumes head dim 128"
        self.T = B * S                 # tokens, b-major
        self.HPC = H // NC             # heads per core
        self.FPC = self.HPC * self.DK  # features per core (q or k or v)
        self.W3 = 3 * self.FPC
        self.DCH = D // 128            # contraction chunks
        self.TT = 512                  # qkv token tile
        self.NTT = self.T // self.TT
        self.TQ = 512                  # attention tq tile
        self.SQT = S // self.TQ        # tq tiles per batch
        self.TPC = self.T // NC        # tokens per core for out-proj
        self.NTI = self.TPC // 128     # out-proj token chunks per core
        self.NJS = D // 512            # out-proj j tiles (512 wide)
        self.SCALE = float(1.0 / np.sqrt(self.DK))


FULL = Cfg(B=2, S=2048, D=2048, H=16, NC=8)


# --------------------------------------------------------------------------
# host-side prep
# --------------------------------------------------------------------------

def host_prep(cfg, x, w_qkv, w_out, cos, sin):
    B, S, D, H, NC = cfg.B, cfg.S, cfg.D, cfg.H, cfg.NC
    DK, T, HPC, FPC = cfg.DK, cfg.T, cfg.HPC, cfg.FPC

    x = np.asarray(x, dtype=np.float32)
    w_qkv = np.asarray(w_qkv, dtype=np.float32)
    w_out = np.asarray(w_out, dtype=np.float32)
    cos = np.asarray(cos, dtype=np.float32)
    sin = np.asarray(sin, dtype=np.float32)

    # xT rearranged so each token tile tt is ONE contiguous [128, DCH*TT]
    # DMA: xP[p, (tt*DCH + dc)*TT + c] = xT[128*dc + p, TT*tt + c]
    xT = x.reshape(T, D).T.astype(np.float16)                      # [D, T]
    NTT, DCH, TT = cfg.NTT, cfg.DCH, cfg.TT
    xP = np.ascontiguousarray(
        xT.reshape(DCH, 128, NTT, TT).transpose(1, 2, 0, 3).reshape(128, T * DCH)
    )

    # per-head [even; odd] feature permutation for q/k
    perm = np.concatenate([np.arange(0, DK, 2), np.arange(1, DK, 2)])
    w_q, w_k, w_v = w_qkv[0:D], w_qkv[D:2 * D], w_qkv[2 * D:3 * D]

    wqkvT = []
    for c in range(NC):
        rows = slice(FPC * c, FPC * (c + 1))
        wq = w_q[rows].reshape(HPC, DK, D)[:, perm, :].reshape(FPC, D)
        wk = w_k[rows].reshape(HPC, DK, D)[:, perm, :].reshape(FPC, D)
        wv = w_v[rows]
        wqkvT.append(np.ascontiguousarray(
            np.concatenate([wq, wk, wv], axis=0).T.astype(np.float16)))

    cosT = np.tile(cos.T, (1, B))                                  # [64, T]
    sinT = np.tile(sin.T, (1, B))
    cosF = np.ascontiguousarray(np.concatenate([cosT, cosT], axis=0), dtype=np.float32)
    # pre-swapped sin so rope's swap folds into partition-shifted copies:
    # swap(z)*[-s;s] == swap(z*[s;-s])
    sinF = np.ascontiguousarray(np.concatenate([sinT, -sinT], axis=0), dtype=np.float32)

    # multiplicative causal masks for S^T diagonal tiles [128, 4*TQ], fp16
    i = np.arange(128)[:, None]
    j = np.arange(cfg.TQ)[None, :]
    masks = np.concatenate(
        [np.where(i <= j - 128 * m, 1.0, 0.0).astype(np.float16) for m in range(4)],
        axis=1,
    )
    masks = np.ascontiguousarray(masks)

    # w_out^T rearranged so each 512-wide j-slice is ONE contiguous DMA:
    # woutP[p, (js*DCH + fc)*512 + c] = woutT[128*fc + p, 512*js + c]
    woutT = w_out.T.astype(np.float16)                             # [D(f), D(j)]
    NJS = cfg.NJS
    woutP = np.ascontiguousarray(
        woutT.reshape(DCH, 128, NJS, 512).transpose(1, 2, 0, 3).reshape(128, D * DCH)
    )

    ones = np.ones((128, 128), dtype=np.float16)
    shared = dict(xT=xP, cosF=cosF, sinF=sinF, masks=masks, ones=ones, woutT=woutP)
    return shared, wqkvT


# --------------------------------------------------------------------------
# device program
# --------------------------------------------------------------------------

def build_program(cfg):
    nc = bacc.Bacc(
        "TRN2",
        target_bir_lowering=False,
        debug=False,
        num_devices=cfg.NC,
    )

    xT_d = nc.dram_tensor("xT", [128, cfg.T * cfg.DCH], F16, kind="ExternalInput").ap()
    wqkvT_d = nc.dram_tensor("wqkvT", [cfg.D, cfg.W3], F16, kind="ExternalInput").ap()
    cosF_d = nc.dram_tensor("cosF", [128, cfg.T], F32, kind="ExternalInput").ap()
    sinF_d = nc.dram_tensor("sinF", [128, cfg.T], F32, kind="ExternalInput").ap()
    masks_d = nc.dram_tensor("masks", [128, 4 * cfg.TQ], F16, kind="ExternalInput").ap()
    ones_d = nc.dram_tensor("ones", [128, 128], F16, kind="ExternalInput").ap()
    woutT_d = nc.dram_tensor("woutT", [128, cfg.D * cfg.DCH], F16, kind="ExternalInput").ap()
    out_d = nc.dram_tensor("out", [cfg.TPC, cfg.D], F16, kind="ExternalOutput").ap()

    with tile.TileContext(nc) as tc:
        _build_body(
            nc, tc, cfg,
            xT_d, wqkvT_d, cosF_d, sinF_d, masks_d, ones_d, woutT_d, out_d,
        )

    nc.compile()
    return nc


def _phase1_qkv_rope(nc, tc, cfg, xT_d, wqkvT_d, cosF_d, sinF_d, q_t, k_t, v_t):
    T, HPC, FPC, W3 = cfg.T, cfg.HPC, cfg.FPC, cfg.W3
    DCH, NTT, TT = cfg.DCH, cfg.NTT, cfg.TT

    with (
        tc.tile_pool(name="wqkv", bufs=1) as wp,
        tc.tile_pool(name="xin", bufs=3) as xp,
        tc.tile_pool(name="csin", bufs=2) as csp,
        tc.tile_pool(name="ropet", bufs=2) as rtp,
        tc.tile_pool(name="pqk", bufs=6, space="PSUM") as pqkp,
        tc.tile_pool(name="pv", bufs=2, space="PSUM") as pvp,
    ):
        # per-dc weight tiles so the first matmul only waits on chunk 0
        w_ts = []
        for dc in range(DCH):
            w_t = wp.tile([128, W3], F16, name=f"w_{dc}")
            nc.scalar.dma_start(w_t[:], wqkvT_d[128 * dc:128 * (dc + 1), :])
            w_ts.append(w_t)

        for tt in range(NTT):
            b = tt // 4
            tl = tt % 4                 # tile index within batch
            # one contiguous DMA per token tile (host pre-arranged); tt=0 is
            # split per-dc so the first matmul starts as early as possible
            xfull = xp.tile([128, DCH * TT], F16, tag="x", name=f"x_{tt}")
            xbase = tt * DCH * TT
            if tt == 0:
                nc.sync.dma_start(xfull[:, 0:TT], xT_d[:, xbase:xbase + TT])
            cos_t = csp.tile([128, TT], F32, tag="cos")
            nc.sync.dma_start(cos_t[:], cosF_d[:, TT * tt:TT * (tt + 1)])
            sin_t = csp.tile([128, TT], F32, tag="sin")
            nc.sync.dma_start(sin_t[:], sinF_d[:, TT * tt:TT * (tt + 1)])
            if tt == 0:
                for dc in range(1, DCH):
                    nc.sync.dma_start(
                        xfull[:, TT * dc:TT * (dc + 1)],
                        xT_d[:, xbase + TT * dc:xbase + TT * (dc + 1)],
                    )
            elif tt == 1:
                # tt=1 split in quarters: the qk-first order consumes x
                # chunks faster than one monolithic 2MB DMA can land
                for q4 in range(4):
                    nc.sync.dma_start(
                        xfull[:, TT * 4 * q4:TT * 4 * (q4 + 1)],
                        xT_d[:, xbase + TT * 4 * q4:xbase + TT * 4 * (q4 + 1)],
                    )
            else:
                nc.sync.dma_start(xfull[:], xT_d[:, xbase:xbase + DCH * TT])
            xts = [xfull[:, TT * dc:TT * (dc + 1)] for dc in range(DCH)]

            nqk = 2 * HPC
            pqs = [pqkp.tile([128, TT], F32, tag="qk", name=f"pq_{tt}_{i}")
                   for i in range(nqk)]
            # two v outputs (ci, ci+1) share one [128,512] PSUM bank
            pvt = [pvp.tile([128, 2 * FPC], F32, tag="v", name=f"pv_{tt}_{i}")
                   for i in range(2)]
            pvs = [pvt[ci // 2][:, FPC * (ci % 2):FPC * (ci % 2 + 1)]
                   for ci in range(4)]

            # all q/k matmuls FIRST so their psum stops land at ~2/3 of the
            # tile's PE window and RoPE (DVE) overlaps the v matmuls below
            for dc in range(DCH):
                first, last = dc == 0, dc == DCH - 1
                for oc in range(nqk):
                    wsl = w_ts[dc][:, 128 * oc:128 * (oc + 1)]
                    nc.tensor.matmul(
                        pqs[oc][:], wsl, xts[dc], start=first, stop=last
                    )
            for dc in range(DCH):
                first, last = dc == 0, dc == DCH - 1
                wv = w_ts[dc][:, 2 * FPC:W3]
                for ci in range(4):
                    lhs = xts[dc][:, 128 * ci:128 * (ci + 1)]
                    # start=True zeroes the WHOLE psum bank, so only the
                    # first group in each shared bank may issue it
                    nc.tensor.matmul(
                        pvs[ci][:], lhs, wv,
                        start=(first and ci % 2 == 0), stop=last,
                        skip_group_check=(ci % 2 == 1),
                    )

            # rope: z' = z*cosF + swap(z)*sinF. swap(z)*s == swap(z*s~) with
            # s~=swap(s); the swap itself is two partition-shifted
            # Activation copies (BIR allows shifted copy, not TensorTensor).
            for oc in range(nqk):
                hc = oc % HPC
                dst = q_t[(hc, b)] if oc < HPC else k_t[(hc, b)]
                z = pqs[oc]
                d0 = TT * tl
                t1 = rtp.tile([128, TT], F32, tag="t1", name=f"t1_{tt}_{oc}")
                u = rtp.tile([128, TT], F32, tag="u", name=f"u_{tt}_{oc}")
                usw = rtp.tile([128, TT], F32, tag="usw", name=f"usw_{tt}_{oc}")
                nc.vector.tensor_mul(t1[:], z[:], cos_t[:])
                nc.vector.tensor_mul(u[:], z[:], sin_t[:])
                nc.scalar.copy(usw[0:64, :], u[64:128, :])
                nc.scalar.copy(usw[64:128, :], u[0:64, :])
                nc.vector.tensor_add(dst[:, d0:d0 + TT], t1[:], usw[:])

            # v: each psum bank is [hc0 g, hc1 g, hc0 g+1, hc1 g+1] matching
            # v_t[b]'s (256*g_local + 128*hc) layout -> ONE copy per bank
            for i in range(2):
                gl = 4 * tl + 2 * i
                nc.scalar.copy(
                    v_t[b][:, 256 * gl:256 * gl + 512], pvt[i][:],
                )


def _phase2_attention(nc, tc, cfg, q_t, k_t, v_t, oT_t, masks_sb, ones_sb,
                      wout_sb, woutT_d, obs, a2a_outs, rhs_gather):
    T, S, HPC, TQ, SQT = cfg.T, cfg.S, cfg.HPC, cfg.TQ, cfg.SQT
    DCH = cfg.DCH

    with (
        tc.tile_pool(name="pexp", bufs=8) as pep,
        tc.tile_pool(name="lacc", bufs=3) as lap,
        tc.tile_pool(name="linv", bufs=2) as lip,
        tc.tile_pool(name="pss", bufs=4, space="PSUM") as pssp,
        tc.tile_pool(name="pso", bufs=2, space="PSUM") as psop,
        tc.tile_pool(name="psl", bufs=2, space="PSUM") as pslp,
    ):
        # preload the full fp16 w_out while attention runs (4 batched DMAs
        # on the Scalar queue; the triggers cost ~30ns each amid the exps)
        for js in range(cfg.NJS):
            nc.scalar.dma_start(
                wout_sb[:, DCH * 512 * js:DCH * 512 * (js + 1)],
                woutT_d[:, DCH * 512 * js:DCH * 512 * (js + 1)],
            )

        onesr = ones_sb[:]

        # The PE executes its queue in order, so an AV matmul waiting on its
        # exp would stall later, already-runnable scores matmuls. Software-
        # pipeline the PE issue order: scores run LOOKAHEAD chunks ahead of
        # the AV matmuls, across block boundaries.
        LOOKAHEAD = 2
        pend_av = []

        def drain_av(keep):
            while len(pend_av) > keep:
                pend_av.pop(0)()

        for hc in range(HPC):
            for b in range(cfg.B):
                for jq in range(SQT):
                    o_ps = psop.tile([128, TQ], F32, tag="o",
                                     name=f"o_{hc}_{b}_{jq}")
                    l_ps = pslp.tile([128, TQ], F32, tag="l",
                                     name=f"l_{hc}_{b}_{jq}")
                    l_acc = lap.tile([128, TQ], F16, tag="la",
                                     name=f"la_{hc}_{b}_{jq}")
                    if jq == 0:
                        nc.vector.memset(l_acc[:], 0.0)
                    qsl = q_t[(hc, b)][:, TQ * jq:TQ * (jq + 1)]
                    # off-diagonal chunks first, then diagonal chunks
                    # m=3,2,1,0 so the full-width m=0 chunk is processed
                    # last and carries stop=True for the whole psum bank
                    cks = list(range(4 * jq)) + [4 * jq + m for m in (3, 2, 1, 0)]
                    for idx, ck in enumerate(cks):
                        m = ck - 4 * jq
                        c0 = 128 * m if m >= 1 else 0
                        last_ck = (m == 0)
                        first_ck = (idx == 0)
                        s_ps = pssp.tile([128, TQ], F32, tag="s",
                                         name=f"s_{hc}_{b}_{jq}_{ck}")
                        ksl = k_t[(hc, b)][:, 128 * ck:128 * (ck + 1)]
                        nc.tensor.matmul(
                            s_ps[:, c0:TQ], ksl, qsl[:, c0:TQ],
                            start=True, stop=True,
                        )
                        p_sb = pep.tile([128, TQ], F16, tag="p",
                                        name=f"p_{hc}_{b}_{jq}_{ck}")
                        nc.scalar.activation(
                            p_sb[:, c0:TQ], s_ps[:, c0:TQ],
                            mybir.ActivationFunctionType.Exp,
                            scale=cfg.SCALE,
                        )
                        if m >= 0:
                            nc.vector.tensor_mul(
                                p_sb[:, c0:TQ], p_sb[:, c0:TQ],
                                masks_sb[:, TQ * m + c0:TQ * (m + 1)],
                            )
                        if first_ck and jq > 0:
                            nc.vector.tensor_copy(l_acc[:], p_sb[:])
                        else:
                            nc.vector.tensor_add(
                                l_acc[:, c0:TQ], l_acc[:, c0:TQ],
                                p_sb[:, c0:TQ],
                            )
                        vsl = v_t[b][:, 256 * ck + 128 * hc:
                                     256 * ck + 128 * (hc + 1)]

                        def av(o_ps=o_ps, vsl=vsl, p_sb=p_sb, c0=c0,
                               first_ck=first_ck, last_ck=last_ck):
                            nc.tensor.matmul(
                                o_ps[:, c0:TQ], vsl, p_sb[:, c0:TQ],
                                start=first_ck, stop=last_ck,
                            )
                        pend_av.append(av)
                        drain_av(LOOKAHEAD)
                    # the omult below must follow the stop-AV in issue order
                    drain_av(0)
                    nc.tensor.matmul(l_ps[:], onesr, l_acc[:],
                                     start=True, stop=True)
                    linv = lip.tile([128, TQ], F32, tag="li",
                                    name=f"li_{hc}_{b}_{jq}")
                    nc.vector.reciprocal_approx_fast(linv[:], l_ps[:])
                    nc.vector.tensor_mul(
                        oT_t[(hc, b)][:, TQ * jq:TQ * (jq + 1)],
                        o_ps[:], linv[:],
                    )
                    # bounce this tile to DRAM now so the head's AllToAll
                    # fires immediately after its last tile
                    s_idx = 4 * b + jq
                    nc.sync.dma_start(
                        obs[hc][128 * s_idx:128 * (s_idx + 1), :],
                        oT_t[(hc, b)][:, TQ * jq:TQ * (jq + 1)],
                    )
            drain_av(0)
            # all 8 tiles of this head are bounced -> fire its AllToAll
            nc.gpsimd.collective_compute(
                "AllToAll",
                mybir.AluOpType.bypass,
                replica_groups=[list(range(cfg.NC))],
                ins=[obs[hc][:].opt()],
                outs=[a2a_outs[hc][:].opt()],
            )
        # gathers issue only now: placed any earlier they head-of-line
        # block either the other head's bounces (Sync) or its exps (Scalar)
        for hc in range(HPC):
            rhs_gather(hc)


def _phase3_outproj(nc, tc, cfg, wout_sb, rhs, out_d):
    HPC, DCH, NJS, NTI = cfg.HPC, cfg.DCH, cfg.NJS, cfg.NTI

    with (
        tc.tile_pool(name="opart", bufs=1) as opp,
        tc.tile_pool(name="osb", bufs=3) as osbp,
        tc.tile_pool(name="pout", bufs=8, space="PSUM") as poutp,
    ):
        fcs_a = [fc for fc in range(DCH) if fc % HPC == 0]   # head 0 features
        fcs_b = [fc for fc in range(DCH) if fc % HPC != 0]   # head 1 features
        tiles = [(js, ti) for js in range(NJS) for ti in range(NTI)]

        # pass A: accumulate ALL head-0 features for every output tile into
        # PSUM, park the partials in SBUF so the banks free up; hides the
        # second AllToAll's rendezvous + wire time
        opart = opp.tile([128, NJS * NTI * 512], F32, name="opart")
        for js, ti in tiles:
            ps = poutp.tile([128, 512], F32, tag="po", name=f"pa_{js}_{ti}")
            for i, fc in enumerate(fcs_a):
                nc.tensor.matmul(
                    ps[:],
                    rhs[fc][:, 128 * ti:128 * (ti + 1)],
                    wout_sb[:, (DCH * js + fc) * 512:
                            (DCH * js + fc) * 512 + 512],
                    start=(i == 0), stop=(i == len(fcs_a) - 1),
                )
            nc.vector.tensor_copy(
                opart[:, (NTI * js + ti) * 512:(NTI * js + ti + 1) * 512],
                ps[:],
            )

        # pass B: head-1 features + parked partial -> fp16 output tile
        for js, ti in tiles:
            ps = poutp.tile([128, 512], F32, tag="po", name=f"pb_{js}_{ti}")
            for i, fc in enumerate(fcs_b):
                nc.tensor.matmul(
                    ps[:],
                    rhs[fc][:, 128 * ti:128 * (ti + 1)],
                    wout_sb[:, (DCH * js + fc) * 512:
                            (DCH * js + fc) * 512 + 512],
                    start=(i == 0), stop=(i == len(fcs_b) - 1),
                )
            osb = osbp.tile([128, 512], F16, tag="ob", name=f"ob_{js}_{ti}")
            nc.vector.tensor_add(
                osb[:], ps[:],
                opart[:, (NTI * js + ti) * 512:(NTI * js + ti + 1) * 512],
            )
            nc.sync.dma_start(
                out_d[128 * ti:128 * (ti + 1), 512 * js:512 * (js + 1)],
                osb[:],
            )


def _build_body(nc, tc, cfg, xT_d, wqkvT_d, cosF_d, sinF_d, masks_d, ones_d,
                woutT_d, out_d):
    T, S, HPC, TQ = cfg.T, cfg.S, cfg.HPC, cfg.TQ

    with tc.tile_pool(name="const", bufs=1) as constp:
        ones_sb = constp.tile([128, 128], F16)
        nc.gpsimd.dma_start(ones_sb[:], ones_d[:])
        masks_sb = constp.tile([128, 4 * TQ], F16)
        nc.gpsimd.dma_start(masks_sb[:], masks_d[:])
        # warm the Exp activation table off the critical path
        warm = constp.tile([128, 1], F32, name="actwarm")
        nc.scalar.activation(
            warm[:], ones_sb[:, 0:1], mybir.ActivationFunctionType.Exp
        )

        with tc.tile_pool(name="qkvp", bufs=1) as qkvp:
            q_t, k_t, oT_t, v_t = {}, {}, {}, {}
            for hc in range(HPC):
                for b in range(cfg.B):
                    q_t[(hc, b)] = qkvp.tile([128, S], F16, name=f"q_{hc}_{b}")
                    k_t[(hc, b)] = qkvp.tile([128, S], F16, name=f"k_{hc}_{b}")
            for b in range(cfg.B):
                v_t[b] = qkvp.tile([128, HPC * S], F16, name=f"v_{b}")

            _phase1_qkv_rope(
                nc, tc, cfg, xT_d, wqkvT_d, cosF_d, sinF_d, q_t, k_t, v_t
            )

            with (
                tc.tile_pool(name="oT", bufs=1) as otp,
                tc.tile_pool(name="dram", bufs=1, space="DRAM") as dramp,
            ):
                for hc in range(HPC):
                    for b in range(cfg.B):
                        oT_t[(hc, b)] = otp.tile(
                            [128, S], F16, name=f"oT_{hc}_{b}")
                wout_sb = otp.tile([128, cfg.DCH * cfg.D], F16)

                obs, a2a_outs = [], []
                for hc in range(HPC):
                    obs.append(dramp.tile(
                        [cfg.NC * 128, cfg.TPC], F16, name=f"obounce{hc}"))
                    a2a_outs.append(dramp.tile(
                        [cfg.NC * 128, cfg.TPC], F16, name=f"a2a_out{hc}"))

                with tc.tile_pool(name="rhsp", bufs=1) as rhsp:
                    rhs_scr = rhsp.tile(
                        [128, cfg.DCH * cfg.TPC], F16, name="rhs_scr")
                    rhs = [None] * cfg.DCH

                    def rhs_gather(hc):
                        for r_ in range(cfg.NC):
                            fc = r_ * HPC + hc
                            sl = rhs_scr[:, cfg.TPC * fc:cfg.TPC * (fc + 1)]
                            nc.sync.dma_start(
                                sl,
                                a2a_outs[hc][128 * r_:128 * (r_ + 1), :])
                            rhs[fc] = sl

                    _phase2_attention(
                        nc, tc, cfg, q_t, k_t, v_t, oT_t, masks_sb, ones_sb,
                        wout_sb, woutT_d, obs, a2a_outs, rhs_gather,
                    )
                    _phase3_outproj(nc, tc, cfg, wout_sb, rhs, out_d)


# --------------------------------------------------------------------------
# host entry point
# --------------------------------------------------------------------------

_CACHE = {}


def _compiled(cfg):
    key = (cfg.B, cfg.S, cfg.D, cfg.H, cfg.NC)
    if key not in _CACHE:
        _CACHE[key] = build_program(cfg)
    return _CACHE[key]


def make_in_maps(cfg, inputs):
    shared, wqkvT = host_prep(
        cfg, inputs["x"], inputs["w_qkv"], inputs["w_out"],
        inputs["cos"], inputs["sin"],
    )
    return [{**shared, "wqkvT": wqkvT[c]} for c in range(cfg.NC)]


def assemble(cfg, results):
    out = np.concatenate([results[c]["out"] for c in range(cfg.NC)], axis=0)
    return out.reshape(cfg.B, cfg.S, cfg.D).astype(np.float32)


def kernel(x, w_qkv, w_out, cos, sin):
    cfg = FULL
    nc = _compiled(cfg)
    in_maps = make_in_maps(cfg, dict(x=x, w_qkv=w_qkv, w_out=w_out, cos=cos, sin=sin))
    res = bass_utils.run_bass_kernel_spmd(nc, in_maps, core_ids=list(range(cfg.NC)))
    return assemble(cfg, res.results)
